# revision 52
# baseline (speedup 1.0000x reference)
"""DCNv2 (offset conv -> bilinear-sampled modulated deform conv) + BN + ReLU
on 8 TRN2 NeuronCores — v2 pipelined.

Per core (data-parallel over the 256 global rows, 32 rows/core):
  - host precomputes x_rows [42x74 pixel-rows, 256ch] bf16 (5-guard-col /
    5-halo-row padded) so the gather sources the DRAM input directly, plus
    the channel-on-partition x_T for the offset conv and bf16 weights.
  - per 4-tile group: offset conv on PE (channel-major [27,512] psum) ->
    per-tile PE transpose -> slim f32 field ops on floor(dy)/floor(dx)
    directly (clip to +-4/+3; guards absorb all out-of-image taps) ->
    int16 idx wrap + DRAM-bounce replication -> gpsimd dma_gather of
    (x0,x1) pairs (1024B descriptors), 18 groups per pixel tile.
  - blend: 36 per-corner tensor_scalar mults (24 DVE / 12 ACT, 4x bf16),
    pair-adds on DVE; PE transposes S chunks into two [128,1024] bf16 psum
    banks (3 rounds), copies to ST4 (DVE/ACT), einsum accumulates in PSUM,
    BN+ReLU fused in the ACT drain, bf16 stores (host casts to f32).
"""

import sys

import numpy as np

sys.path.insert(0, "/opt/trn_rl_repo")

import concourse.bacc as bacc
import concourse.bass as bass
import concourse.mybir as mybir
from concourse.bass_utils import run_bass_kernel_spmd
from concourse.library_config import mlp
from contextlib import ExitStack

F32 = mybir.dt.float32
BF16 = mybir.dt.bfloat16
I16 = mybir.dt.int16
I32 = mybir.dt.int32
ALU = mybir.AluOpType
ACTF = mybir.ActivationFunctionType

B, H, W, C, F = 4, 64, 64, 256, 256
K = 9
NCORES = 8
RPC = (B * H) // NCORES      # 32 output rows per core
P = RPC * W                  # 2048 pixels per core
NT = P // 128                # 16 pixel tiles
NG = 4                       # tile groups (4 tiles = 8 rows each)
HALO = 5                     # rows of halo each side
RIN = RPC + 2 * HALO         # 42 stored rows
GUARD = 5                    # zero guard cols each side
WP = W + 2 * GUARD           # 74 stored cols
NPIX = RIN * WP              # 3108 x_rows pixel-rows
CT = 34                      # conv x_T rows (-1 .. 32)
CW = 66                      # conv x_T cols (-1 .. 64)
BN_EPS = 1e-3

KY = np.array([-1, -1, -1, 0, 0, 0, 1, 1, 1], np.float32)
KX = np.array([-1, 0, 1, -1, 0, 1, -1, 0, 1], np.float32)

# ST4 transpose copy rounds: (first chunk, n chunks); engines A, A, D
ROUNDS = [(0, 8), (8, 8), (16, 2)]
RND_ENG = ["A", "A", "D"]
NDVE_C = 25  # corners (of 36) whose blend mults run on DVE; rest on ACT


def cp_counts_upto(gr):
    """(#ACT rounds, #DVE rounds) among global rounds < gr."""
    a = d = 0
    for x in range(gr):
        if RND_ENG[x % 3] == "A":
            a += 1
        else:
            d += 1
    return a, d


def build_graph():
    nc = bacc.Bacc("TRN2")
    # same-engine RAW chains are ordered by the in-order engines; cross-engine
    # hazards are covered by semaphores below.
    nc.detect_race_conditions = False

    x_rows = nc.declare_dram_parameter("x_rows", [NPIX, C], BF16, isOutput=False)
    x_t0 = nc.declare_dram_parameter("x_t0", [128, CT * CW], BF16, isOutput=False)
    x_t1 = nc.declare_dram_parameter("x_t1", [128, CT * CW], BF16, isOutput=False)
    offw = nc.declare_dram_parameter("offw", [128, 18 * 27], BF16, isOutput=False)
    dcnw = nc.declare_dram_parameter("dcnw", [128, 18 * 256], BF16, isOutput=False)
    bnp = nc.declare_dram_parameter("bn", [128, 8], F32, isOutput=False)
    idxb = nc.declare_dram_parameter("idxb", [128, NT * K], F32, isOutput=False)
    ident = nc.declare_dram_parameter("ident", [128, 128], BF16, isOutput=False)
    identf = nc.declare_dram_parameter("identf", [32, 32], F32, isOutput=False)
    out = nc.declare_dram_parameter("out", [2, 128, P], BF16, isOutput=True)

    idx_dram = nc.dram_tensor("idx_dram", [16, NT * 144], I16)

    stack = ExitStack()

    def sb(name, shape, dt):
        return stack.enter_context(nc.sbuf_tensor(name, shape, dt))

    xt_sb = [sb("xt0_sb", [128, CT * CW], BF16), sb("xt1_sb", [128, CT * CW], BF16)]
    offw_sb = sb("offw_sb", [128, 18 * 27], BF16)
    wt_sb = sb("wt_sb", [128, 18 * 256], BF16)
    bn_sb = sb("bn_sb", [128, 8], F32)
    idxb_sb = sb("idxb_sb", [128, NT * K], F32)
    idb_sb = sb("idb_sb", [128, 128], BF16)
    idf_sb = sb("idf_sb", [32, 32], F32)
    off_cm = sb("off_cm", [32, 512], F32)       # [27, 512] used
    off_pix = sb("off_pix", [128, NT * 27], F32)
    m_sb = sb("m_sb", [128, NT * K], F32)
    # field scratch (per-group [128, 36])
    T8 = sb("T8", [128, 36], F32)
    I32A = sb("I32A", [128, 36], I32)
    F8 = sb("F8", [128, 36], F32)
    GT = sb("GT", [128, 36], F32)
    FLY = sb("FLY", [128, 36], F32)
    FLX = sb("FLX", [128, 36], F32)
    FY = sb("FY", [128, 36], F32)
    FX = sb("FX", [128, 36], F32)
    SY8 = sb("SY8", [128, 36], F32)
    SX8 = sb("SX8", [128, 36], F32)
    U1 = sb("U1", [128, 36], F32)
    U0 = sb("U0", [128, 36], F32)
    I0T = sb("I0T", [128, 36], F32)
    s36 = sb("s36", [128, NT * 36], F32)
    idxf = sb("idxf", [128, NT * 18], F32)
    hop1 = sb("hop1", [16, 8 * NT * 18], F32)
    idxs_sb = sb("idxs_sb", [128, NT * 144], I16)
    VBIG = sb("VBIG", [128, 6 * 18 * 512], BF16)  # 6-slot V ring
    S0 = sb("S0", [128, 2304], BF16)
    S1 = sb("S1", [128, 2304], BF16)
    ST4 = [sb("ST4a", [128, 18 * 512], BF16), sb("ST4b", [128, 18 * 512], BF16)]
    out_sb = sb("out_sb", [128, 2 * P], BF16)

    Vb = [VBIG[:, i * 9216 : (i + 1) * 9216] for i in range(6)]
    # gather units: first two tiles single (fast pipeline fill), then pairs
    UNITS = [(0,), (1,)] + [(t, t + 1) for t in range(2, NT, 2)]

    def unit_of(t):
        return t if t < 2 else 2 + (t - 2) // 2
    Sb = [S0, S1]
    out_sb_v = out_sb[:].rearrange("p (h n) -> p h n", h=2)

    def st4_dst(tt, c0, nch):
        return ST4[(tt // 4) % 2][:].rearrange("p (c n) -> p c n", n=512)[
            :, c0 : c0 + nch, (tt % 4) * 128 : (tt % 4) * 128 + 128
        ]

    def sem(name):
        return stack.enter_context(nc.semaphore(name))

    d_in = sem("d_in")       # input loads: 10 DMAs x16
    d_h1 = sem("d_h1")       # hop DMAs: 8/group x16
    d_rep = sem("d_rep")     # bounce DMAs: 2/group x16
    d_out = sem("d_out")
    g_sem = sem("g_sem")     # gather completions x16
    p_sem = sem("p_sem")     # gather preps
    pe_conv = sem("pe_conv")  # 1/group
    pe_offt = sem("pe_offt")  # 1/tile
    pe_tr = sem("pe_tr")     # 1/round (3/tile)
    pe_mm = sem("pe_mm")     # 2/group
    a_cm = sem("a_cm")       # 1/group off_cm copy
    a_off = sem("a_off")     # 1/tile off_pix copy
    a_sig = sem("a_sig")     # 1/group sigmoid
    a_mul = sem("a_mul")     # 1/tile ACT blend mults
    a_cp = sem("a_cp")       # ACT ST4 rounds
    a_bn = sem("a_bn")       # 2/group
    v_fld = sem("v_fld")     # 1/group fields (s36+idxf ready)
    v_i16 = sem("v_i16")     # 1/group idx cast
    v_add = sem("v_add")     # 1/tile S ready
    v_cp = sem("v_cp")       # DVE ST4 rounds

    NLOAD = 8

    blk = stack.enter_context(nc.Block())

    with nc.psum_tensor("ps_off0", [32, 512], F32) as ps_off0, nc.psum_tensor(
        "ps_off1", [32, 512], F32
    ) as ps_off1, nc.psum_tensor(
        "ps_tr0", [128, 1024], BF16
    ) as ptr0, nc.psum_tensor(
        "ps_tr1", [128, 1024], BF16
    ) as ptr1, nc.psum_tensor(
        "ps_e0", [128, 1024], F32
    ) as pe0, nc.psum_tensor(
        "ps_e1", [128, 1024], F32
    ) as pe1:
        ps_off = [ps_off0, ps_off1]
        ps_tr = [ptr0, ptr1]
        ps_e = [pe0, pe1]  # [G%2] -> [128, (h, 512)]

        # =================== SYNC (SP): loads, idx plumbing, stores =========
        @blk.sync
        def _(sync):
            # load order matters: PE conv waits d_in>=80 (first 5), DVE fields
            # wait >=96 (through idxb), ACT blends wait >=112 (through bn)
            sync.dma_start(xt_sb[0][:], x_t0[:]).then_inc(d_in, 16)
            sync.dma_start(xt_sb[1][:], x_t1[:]).then_inc(d_in, 16)
            sync.dma_start(offw_sb[:], offw[:]).then_inc(d_in, 16)
            sync.dma_start(idf_sb[:], identf[:]).then_inc(d_in, 16)
            sync.dma_start(idxb_sb[:], idxb[:]).then_inc(d_in, 16)
            sync.dma_start(idb_sb[:], ident[:]).then_inc(d_in, 16)
            sync.dma_start(bn_sb[:], bnp[:]).then_inc(d_in, 16)
            sync.dma_start(wt_sb[:], dcnw[:]).then_inc(d_in, 16)
            for g in range(NG):
                sync.wait_ge(v_fld, g + 1)
                for s in range(8):
                    sync.dma_start(
                        hop1[:, (s * NT * 18) + g * 72 : (s * NT * 18) + (g + 1) * 72],
                        idxf[s * 16 : (s + 1) * 16, g * 72 : (g + 1) * 72],
                    ).then_inc(d_h1, 16)
                sync.wait_ge(v_i16, g + 1)
                sync.dma_start(
                    idx_dram[:, g * 576 : (g + 1) * 576],
                    idxs_sb[0:16, g * 576 : (g + 1) * 576],
                ).then_inc(d_rep, 16)
                sync.wait_ge(d_rep, 32 * g + 16)
                sync.dma_start(
                    idxs_sb[:, g * 576 : (g + 1) * 576],
                    bass.AP(
                        idx_dram,
                        g * 576,
                        [[0, 8], [NT * 144, 16], [1, 576]],
                    ),
                ).then_inc(d_rep, 16)
            for G in range(NG):
                for h in range(2):
                    sync.wait_ge(a_bn, G * 2 + h + 1)
                    sync.dma_start(
                        out[h, :, G * 512 : (G + 1) * 512],
                        out_sb_v[:, h, G * 512 : (G + 1) * 512],
                    ).then_inc(d_out, 16)
            sync.wait_ge(d_out, 16 * 8)

        # =================== GPSIMD: gathers ===================
        @blk.gpsimd
        def _(gp):
            gp.load_library(mlp)

            def prep(u):
                unit = UNITS[u]
                t0, n = unit[0], len(unit)
                gp.wait_ge(d_rep, 32 * (t0 // 4) + 32)
                gp.dma_gather(
                    VBIG[:, (t0 % 6) * 9216 : (t0 % 6 + n) * 9216].rearrange(
                        "p (g c) -> p g c", c=512
                    ),
                    bass.AP(x_rows, 0, [[256, NPIX - 1], [1, 512]]),
                    idxs_sb[:, t0 * 144 : (t0 + n) * 144],
                    n * 18 * 128,
                    n * 18 * 128,
                    512,
                    elem_step=256,
                    single_packet=False,
                    prepare_only=True,
                    sem=g_sem,
                ).then_inc(p_sem, 1)

            prep(0)
            for u in range(len(UNITS)):
                unit = UNITS[u]
                gp.wait_ge(p_sem, u + 1)
                need = unit[0] + len(unit) - 6
                if need > 0:
                    gp.wait_ge(v_add, need)
                gp.trigger_dma(1)
                if u + 1 < len(UNITS):
                    prep(u + 1)

        # =================== PE ===================
        @blk.tensor
        def _(te):
            te.wait_ge(d_in, 4 * 16)
            # offset convs + off transposes per group (transposes land in the
            # not-yet-used ps_e0 bank; einsum G0 starts long after)
            def offtr(g):
                te.wait_ge(a_cm, g + 1)
                for q in range(4):
                    t = g * 4 + q
                    if t >= 2:
                        te.wait_ge(a_off, t - 1)  # ps_e0 slice free
                    te.transpose(
                        ps_e[0][:, (t % 2) * 32 : (t % 2) * 32 + 27].bitcast(F32),
                        off_cm[0:27, q * 128 : (q + 1) * 128],
                        idf_sb[0:27, 0:27],
                    ).then_inc(pe_offt, 1)

            for g in range(NG):
                if g >= 2:
                    te.wait_ge(a_cm, g - 1)  # ps_off[g%2] free
                ins = None
                for ch in range(18):
                    kk, half = ch // 2, ch % 2
                    ky, kx = kk // 3 - 1, kk % 3 - 1
                    rhs = xt_sb[half][:].rearrange("p (r w) -> p r w", w=CW)[
                        :, g * 8 + ky + 1 : g * 8 + ky + 9, kx + 1 : kx + 65
                    ]
                    ins = te.matmul(
                        ps_off[g % 2][0:27, :],
                        offw_sb[:, ch * 27 : (ch + 1) * 27],
                        rhs,
                        start=(ch == 0),
                        stop=(ch == 17),
                        skip_group_check=True,
                    )
                ins.then_inc(pe_conv, 1)
                offtr(g)
            # S transposes + einsum; offtr(3) slots in after tile 0's rounds
            # to break the ACT go3 <-> PE in-order cycle
            te.wait_ge(d_in, 6 * 16)
            for t in range(NT):
                te.wait_ge(v_add, t + 1)
                S = Sb[t % 2]
                for r, (c0, nch) in enumerate(ROUNDS):
                    gr = t * 3 + r
                    if gr >= 2:
                        a_need, d_need = cp_counts_upto(gr - 1)
                        if RND_ENG[(gr - 2) % 3] == "A":
                            te.wait_ge(a_cp, a_need)
                        else:
                            te.wait_ge(v_cp, d_need)
                    bank = ps_tr[gr % 2]
                    ins = None
                    for j in range(nch):
                        c = c0 + j
                        ins = te.transpose(
                            bank[:, j * 128 : (j + 1) * 128],
                            S[:, c * 128 : (c + 1) * 128],
                            idb_sb[:],
                        )
                    ins.then_inc(pe_tr, 1)
                if 4 <= t < 12:
                    # einsum chunk k of group G = t//4 - 1 (9 matmuls),
                    # interleaved with this group's rounds (ST4 dbl-buffered)
                    G = t // 4 - 1
                    k = t % 4
                    if k == 0:
                        a_need, d_need = cp_counts_upto((4 * G + 4) * 3)
                        te.wait_ge(a_cp, a_need)
                        te.wait_ge(v_cp, d_need)
                    h, chalf = k // 2, k % 2
                    ins = None
                    for j in range(9):
                        c = chalf * 9 + j
                        ins = te.matmul(
                            ps_e[G % 2][:, h * 512 : (h + 1) * 512],
                            wt_sb[:, c * 256 + h * 128 : c * 256 + (h + 1) * 128],
                            ST4[G % 2][:, c * 512 : (c + 1) * 512],
                            start=(c == 0),
                            stop=(c == 17),
                            skip_group_check=True,
                        )
                    if k % 2 == 1:
                        ins.then_inc(pe_mm, 1)
                if t == 11:
                    # whole einsum for G2 right after t11 rounds
                    a_need, d_need = cp_counts_upto(12 * 3)
                    te.wait_ge(a_cp, a_need)
                    te.wait_ge(v_cp, d_need)
                    te.wait_ge(a_bn, 2)  # ps_e[0] free (G0 drained)
                    for h in range(2):
                        ins = None
                        for c in range(18):
                            ins = te.matmul(
                                ps_e[0][:, h * 512 : (h + 1) * 512],
                                wt_sb[:, c * 256 + h * 128 : c * 256 + (h + 1) * 128],
                                ST4[0][:, c * 512 : (c + 1) * 512],
                                start=(c == 0),
                                stop=(c == 17),
                                skip_group_check=True,
                            )
                        ins.then_inc(pe_mm, 1)
                if t // 4 == 3:
                    # last group: per-tile einsum chains so the tail only
                    # exposes tile 15's matmuls
                    q = t % 4
                    a_need, d_need = cp_counts_upto((t + 1) * 3)
                    te.wait_ge(a_cp, a_need)
                    te.wait_ge(v_cp, d_need)
                    if q == 0:
                        te.wait_ge(a_bn, 4)  # ps_e[1] free (G1 drained)
                    for h in range(2):
                        ins = None
                        for c in range(18):
                            ins = te.matmul(
                                ps_e[1][:, h * 512 + q * 128 : h * 512 + (q + 1) * 128],
                                wt_sb[:, c * 256 + h * 128 : c * 256 + (h + 1) * 128],
                                ST4[1][:, c * 512 + q * 128 : c * 512 + (q + 1) * 128],
                                start=(c == 0),
                                stop=(c == 17),
                                skip_group_check=True,
                            )
                        if q == 3:
                            ins.then_inc(pe_mm, 1)

        # =================== DVE ===================
        @blk.vector
        def _(v):
            v.wait_ge(d_in, 6 * 16)

            def fields(g):
                # dy/dx/m views for this group's 4 tiles
                dyv = off_pix[:].rearrange("p (t m) -> p t m", m=27)[
                    :, g * 4 : (g + 1) * 4, 0:9
                ]
                dxv = off_pix[:].rearrange("p (t m) -> p t m", m=27)[
                    :, g * 4 : (g + 1) * 4, 9:18
                ]
                mv = m_sb[:, g * 36 : (g + 1) * 36]
                v.wait_ge(a_off, 4 * (g + 1))
                # floor(dy)
                v.tensor_scalar(T8[:], dyv, 8.0, None, ALU.add)
                v.tensor_copy(I32A[:], T8[:])
                v.tensor_copy(F8[:], I32A[:])
                v.tensor_tensor(GT[:], F8[:], T8[:], ALU.is_gt)
                v.tensor_tensor(FLY[:], F8[:], GT[:], ALU.subtract)
                v.scalar_tensor_tensor(FY[:], dyv, 8.0, FLY[:], ALU.add, ALU.subtract)
                v.tensor_scalar(SY8[:], FLY[:], 11.0, 4.0, ALU.min, ALU.max)
                # floor(dx)
                v.tensor_scalar(T8[:], dxv, 8.0, None, ALU.add)
                v.tensor_copy(I32A[:], T8[:])
                v.tensor_copy(F8[:], I32A[:])
                v.tensor_tensor(GT[:], F8[:], T8[:], ALU.is_gt)
                v.tensor_tensor(FLX[:], F8[:], GT[:], ALU.subtract)
                v.scalar_tensor_tensor(FX[:], dxv, 8.0, FLX[:], ALU.add, ALU.subtract)
                v.tensor_scalar(SX8[:], FLX[:], 11.0, 4.0, ALU.min, ALU.max)
                # idx: (SY8*74 + SX8) + IDXB8 ; idx1 = idx0 + 74
                v.scalar_tensor_tensor(I0T[:], SY8[:], 74.0, SX8[:], ALU.mult, ALU.add)
                # idx table group order must match V blend order: g = k*2 + yc
                idxf_v = idxf[:].rearrange("p (t k2 g2) -> p t k2 g2", k2=9, g2=2)
                ib_v = idxb_sb[:].rearrange("p (t k) -> p t k", k=9)[
                    :, g * 4 : (g + 1) * 4, :
                ]
                v.tensor_tensor(
                    idxf_v[:, g * 4 : (g + 1) * 4, :, 0], I0T[:], ib_v, ALU.add
                )
                v.tensor_scalar(
                    idxf_v[:, g * 4 : (g + 1) * 4, :, 1],
                    idxf_v[:, g * 4 : (g + 1) * 4, :, 0],
                    74.0,
                    None,
                    ALU.add,
                )
                # blend scalars: s[(k,yc),xc]; u1 = fy*m, u0 = m-u1
                v.wait_ge(a_sig, g + 1)
                v.tensor_tensor(U1[:], FY[:], mv, ALU.mult)
                v.tensor_tensor(U0[:], mv, U1[:], ALU.subtract)
                s_v = s36[:].rearrange("p (t k yc xc) -> p t k yc xc", k=9, yc=2, xc=2)[
                    :, g * 4 : (g + 1) * 4
                ]
                u0_v = U0[:].rearrange("p (t k) -> p t k", k=9)
                u1_v = U1[:].rearrange("p (t k) -> p t k", k=9)
                fx_v = FX[:].rearrange("p (t k) -> p t k", k=9)
                v.tensor_tensor(s_v[:, :, :, 0, 1], u0_v, fx_v, ALU.mult)
                v.tensor_tensor(s_v[:, :, :, 0, 0], u0_v, s_v[:, :, :, 0, 1], ALU.subtract)
                v.tensor_tensor(s_v[:, :, :, 1, 1], u1_v, fx_v, ALU.mult)
                v.tensor_tensor(
                    s_v[:, :, :, 1, 0], u1_v, s_v[:, :, :, 1, 1], ALU.subtract
                ).then_inc(v_fld, 1)
                # int16 idx wrap (after hop DMAs)
                v.wait_ge(d_h1, 128 * (g + 1))
                v.tensor_copy(
                    idxs_sb[0:16, g * 576 : (g + 1) * 576].rearrange(
                        "q (t g2 s) -> q t g2 s", t=4, g2=18
                    ),
                    hop1[:].rearrange("q (s t g2) -> q t g2 s", s=8, t=NT)[
                        :, g * 4 : (g + 1) * 4
                    ],
                ).then_inc(v_i16, 1)

            def blend(t):
                v.wait_ge(g_sem, 16 * (unit_of(t) + 1))
                V = Vb[t % 6]
                Vv = V.rearrange("p (k yc xc c) -> p k yc xc c", yc=2, xc=2, c=256)
                for ci in range(NDVE_C):
                    g18, xc = ci // 2, ci % 2
                    k, yc = g18 // 2, g18 % 2
                    col = t * 36 + (k * 2 + yc) * 2 + xc
                    v.tensor_scalar(
                        Vv[:, k, yc, xc, :],
                        Vv[:, k, yc, xc, :],
                        s36[:, col : col + 1],
                        None,
                        ALU.mult,
                    )
                v.wait_ge(a_mul, t + 1)
                # H = Vx0 + Vx1 (in place into xc0), S = H(yc0) + H(yc1)
                Vf = V.rearrange("p (g n) -> p g n", n=512)
                v.tensor_tensor(
                    Vf[:, :, 0:256], Vf[:, :, 0:256], Vf[:, :, 256:512], ALU.add
                )
                if t >= 2:
                    v.wait_ge(pe_tr, 3 * (t - 1))  # S[t%2] free
                Vp = V.rearrange("p (k yc n) -> p k yc n", yc=2, n=512)
                S = Sb[t % 2][:].rearrange("p (k c) -> p k c", c=256)
                v.tensor_tensor(
                    S, Vp[:, :, 0, 0:256], Vp[:, :, 1, 0:256], ALU.add
                ).then_inc(v_add, 1)
                # ST4 copy round 2 of tile t-1
                if t >= 1:
                    tt = t - 1
                    gr = tt * 3 + 2
                    v.wait_ge(pe_tr, gr + 1)
                    if tt >= 8:
                        v.wait_ge(pe_mm, 2 * (tt // 4) - 2)
                    c0, nch = ROUNDS[2]
                    v.tensor_copy(
                        st4_dst(tt, c0, nch),
                        ps_tr[gr % 2][:].rearrange("p (c n) -> p c n", n=128)[
                            :, 0:nch, :
                        ],
                    ).then_inc(v_cp, 1)

            fields(0)
            fields(1)
            fields(2)
            fields(3)
            for t in range(NT):
                blend(t)
            # drain tile 15 round 2
            tt = NT - 1
            gr = tt * 3 + 2
            v.wait_ge(pe_tr, gr + 1)
            c0, nch = ROUNDS[2]
            v.tensor_copy(
                st4_dst(tt, c0, nch),
                ps_tr[gr % 2][:].rearrange("p (c n) -> p c n", n=128)[:, 0:nch, :],
            ).then_inc(v_cp, 1)

        # =================== ACT ===================
        @blk.scalar
        def _(a):
            def group_off(g):
                a.wait_ge(pe_conv, g + 1)
                a.copy(off_cm[0:27, :], ps_off[g % 2][0:27, :]).then_inc(a_cm, 1)
                for q in range(4):
                    t = g * 4 + q
                    a.wait_ge(pe_offt, t + 1)
                    a.copy(
                        off_pix[:].rearrange("p (t m) -> p t m", m=27)[:, t, :],
                        ps_e[0][:, (t % 2) * 32 : (t % 2) * 32 + 27].bitcast(F32),
                    ).then_inc(a_off, 1)
                a.activation(
                    m_sb[:, g * 36 : (g + 1) * 36],
                    off_pix[:].rearrange("p (t m) -> p t m", m=27)[
                        :, g * 4 : (g + 1) * 4, 18:27
                    ],
                    ACTF.Sigmoid,
                ).then_inc(a_sig, 1)

            def blend_a(t):
                a.wait_ge(g_sem, 16 * (unit_of(t) + 1))
                a.wait_ge(v_fld, t // 4 + 1)
                V = Vb[t % 6]
                Vv = V.rearrange("p (k yc xc c) -> p k yc xc c", yc=2, xc=2, c=256)
                last = None
                for ci in range(NDVE_C, 36):
                    g18, xc = ci // 2, ci % 2
                    k, yc = g18 // 2, g18 % 2
                    col = t * 36 + (k * 2 + yc) * 2 + xc
                    last = a.mul(
                        Vv[:, k, yc, xc, :],
                        Vv[:, k, yc, xc, :],
                        s36[:, col : col + 1],
                    )
                last.then_inc(a_mul, 1)
                # ST4 copy rounds 0, 1 of tile t-1
                if t >= 1:
                    tt = t - 1
                    for r in (0, 1):
                        gr = tt * 3 + r
                        a.wait_ge(pe_tr, gr + 1)
                        if r == 0 and tt >= 8:
                            a.wait_ge(pe_mm, 2 * (tt // 4) - 2)
                        c0, nch = ROUNDS[r]
                        a.copy(
                            st4_dst(tt, c0, nch),
                            ps_tr[gr % 2][:].rearrange("p (c n) -> p c n", n=128)[
                                :, 0:nch, :
                            ],
                        ).then_inc(a_cp, 1)
                G_bn = {8: 0, 12: 1, 13: 2}.get(t)
                if G_bn is not None:
                    G = G_bn
                    for h in range(2):
                        a.wait_ge(pe_mm, 2 * G + h + 1)
                        a.activation(
                            out_sb_v[:, h, G * 512 : (G + 1) * 512],
                            ps_e[G % 2][:, h * 512 : (h + 1) * 512],
                            ACTF.Relu,
                            bias=bn_sb[:, 2 + h : 3 + h],
                            scale=bn_sb[:, h : h + 1],
                        ).then_inc(a_bn, 1)

            group_off(0)
            group_off(1)
            group_off(2)
            group_off(3)
            for t in range(NT):
                blend_a(t)
            # drain: tile 15 rounds 0, 1, then einsum G3 BN
            tt = NT - 1
            for r in (0, 1):
                gr = tt * 3 + r
                a.wait_ge(pe_tr, gr + 1)
                c0, nch = ROUNDS[r]
                a.copy(
                    st4_dst(tt, c0, nch),
                    ps_tr[gr % 2][:].rearrange("p (c n) -> p c n", n=128)[:, 0:nch, :],
                ).then_inc(a_cp, 1)
            for G in (3,):
                for h in range(2):
                    a.wait_ge(pe_mm, 2 * G + h + 1)
                    a.activation(
                        out_sb_v[:, h, G * 512 : (G + 1) * 512],
                        ps_e[G % 2][:, h * 512 : (h + 1) * 512],
                        ACTF.Relu,
                        bias=bn_sb[:, 2 + h : 3 + h],
                        scale=bn_sb[:, h : h + 1],
                    ).then_inc(a_bn, 1)

    stack.close()
    if not nc.is_finalized():
        nc.finalize()
    return nc


def _host_consts():
    import ml_dtypes

    p = np.arange(128)
    r = p // 64  # row within tile-pair
    c = p % 64
    idxb8 = np.zeros((128, NT, K), np.float32)
    for t in range(NT):
        rho = t * 2 + r  # local output row 0..31
        for k in range(K):
            base = (rho + HALO + KY[k]) * WP + (c + GUARD + KX[k])
            idxb8[:, t, k] = base - (8 * 74 + 8)
    ident = np.eye(128, dtype=ml_dtypes.bfloat16)
    identf = np.eye(32, dtype=np.float32)
    return idxb8.reshape(128, NT * K), np.asarray(ident), identf


def make_in_maps(x, offset_w, dcn_w, gamma, beta, moving_mean, moving_var):
    import ml_dtypes

    x = np.ascontiguousarray(x, np.float32)
    idxb8, ident, identf = _host_consts()

    # offw [128, 18*27]: row (kk*256+cin) -> [cin%128, (kk*2+cin//128)*27+m]
    ow = np.asarray(offset_w, np.float32).reshape(18, 128, 27)
    offw_h = np.ascontiguousarray(
        np.transpose(ow, (1, 0, 2)).reshape(128, 18 * 27).astype(ml_dtypes.bfloat16)
    )
    dw = np.asarray(dcn_w, np.float32).reshape(18, 128, 256)
    dcnw_h = np.ascontiguousarray(
        np.transpose(dw, (1, 0, 2)).reshape(128, 18 * 256).astype(ml_dtypes.bfloat16)
    )

    inv_f = np.asarray(gamma, np.float32) / np.sqrt(
        np.asarray(moving_var, np.float32) + BN_EPS
    )
    ab_f = np.asarray(beta, np.float32) - np.asarray(moving_mean, np.float32) * inv_f
    bn_h = np.zeros((128, 8), np.float32)
    for h in range(2):
        bn_h[:, h] = inv_f.reshape(2, 128)[h]
        bn_h[:, 2 + h] = ab_f.reshape(2, 128)[h]

    in_maps = []
    for core in range(NCORES):
        r0 = core * RPC
        b = r0 // H
        rb = r0 % H
        # padded rows rb-HALO .. rb+RPC+HALO+1 (for conv we need rb-1..rb+32)
        pad = np.zeros((RIN, WP, C), np.float32)
        lo = rb - HALO
        hi = rb + RPC + HALO
        slo, shi = max(lo, 0), min(hi, H)
        pad[slo - lo : shi - lo, GUARD : GUARD + W] = x[b, slo:shi]
        x_rows_h = np.ascontiguousarray(
            pad.reshape(RIN * WP, C).astype(ml_dtypes.bfloat16)
        )
        # conv x_T: rows rb-1 .. rb+32 (34), cols -1..64 (66), ch-on-partition
        conv_rows = pad[HALO - 1 : HALO - 1 + CT, GUARD - 1 : GUARD - 1 + CW]
        x_t = np.transpose(conv_rows, (2, 0, 1)).reshape(C, CT * CW)
        x_t16 = x_t.astype(ml_dtypes.bfloat16)
        in_maps.append(
            dict(
                x_rows=x_rows_h,
                x_t0=np.ascontiguousarray(x_t16[0:128]),
                x_t1=np.ascontiguousarray(x_t16[128:256]),
                offw=offw_h,
                dcnw=dcnw_h,
                bn=bn_h,
                idxb=idxb8,
                ident=ident,
                identf=identf,
            )
        )
    return in_maps


def kernel(x, offset_w, dcn_w, gamma, beta, moving_mean, moving_var):
    in_maps = make_in_maps(
        x, offset_w, dcn_w, gamma, beta, moving_mean, moving_var
    )
    nc = build_graph()
    res = run_bass_kernel_spmd(nc, in_maps, list(range(NCORES)))
    outs = res.results if hasattr(res, "results") else res

    full = np.zeros((B, H, W, F), np.float32)
    for core in range(NCORES):
        o = np.asarray(outs[core]["out"]).astype(np.float32)  # [2, 128, P]
        o = o.reshape(256, P).T.reshape(RPC, W, F)
        r0 = core * RPC
        full[r0 // H, r0 % H : r0 % H + RPC] = o
    return full


if __name__ == "__main__":
    import reference

    inp = {k: np.asarray(v) for k, v in reference.setup_inputs().items()}
    got = kernel(**inp)
    print("kernel ran, shape", got.shape)


# revision 53
# speedup vs baseline: 1.0352x; 1.0352x over previous
"""DCNv2 (offset conv -> bilinear-sampled modulated deform conv) + BN + ReLU
on 8 TRN2 NeuronCores — v2 pipelined.

Per core (data-parallel over the 256 global rows, 32 rows/core):
  - host precomputes x_rows [42x74 pixel-rows, 256ch] bf16 (5-guard-col /
    5-halo-row padded) so the gather sources the DRAM input directly, plus
    the channel-on-partition x_T for the offset conv and bf16 weights.
  - per 4-tile group: offset conv on PE (channel-major [27,512] psum) ->
    per-tile PE transpose -> slim f32 field ops on floor(dy)/floor(dx)
    directly (clip to +-4/+3; guards absorb all out-of-image taps) ->
    int16 idx wrap + DRAM-bounce replication -> gpsimd dma_gather of
    (x0,x1) pairs (1024B descriptors), 18 groups per pixel tile.
  - blend: 36 per-corner tensor_scalar mults (24 DVE / 12 ACT, 4x bf16),
    pair-adds on DVE; PE transposes S chunks into two [128,1024] bf16 psum
    banks (3 rounds), copies to ST4 (DVE/ACT), einsum accumulates in PSUM,
    BN+ReLU fused in the ACT drain, bf16 stores (host casts to f32).
"""

import sys

import numpy as np

sys.path.insert(0, "/opt/trn_rl_repo")

import concourse.bacc as bacc
import concourse.bass as bass
import concourse.mybir as mybir
from concourse.bass_utils import run_bass_kernel_spmd
from concourse.library_config import mlp
from contextlib import ExitStack

F32 = mybir.dt.float32
BF16 = mybir.dt.bfloat16
I16 = mybir.dt.int16
I32 = mybir.dt.int32
ALU = mybir.AluOpType
ACTF = mybir.ActivationFunctionType

B, H, W, C, F = 4, 64, 64, 256, 256
K = 9
NCORES = 8
RPC = (B * H) // NCORES      # 32 output rows per core
P = RPC * W                  # 2048 pixels per core
NT = P // 128                # 16 pixel tiles
NG = 4                       # tile groups (4 tiles = 8 rows each)
HALO = 5                     # rows of halo each side
RIN = RPC + 2 * HALO         # 42 stored rows
GUARD = 5                    # zero guard cols each side
WP = W + 2 * GUARD           # 74 stored cols
NPIX = RIN * WP              # 3108 x_rows pixel-rows
CT = 34                      # conv x_T rows (-1 .. 32)
CW = 66                      # conv x_T cols (-1 .. 64)
BN_EPS = 1e-3

KY = np.array([-1, -1, -1, 0, 0, 0, 1, 1, 1], np.float32)
KX = np.array([-1, 0, 1, -1, 0, 1, -1, 0, 1], np.float32)

# ST4 transpose copy rounds: (first chunk, n chunks); engines A, A, D
ROUNDS = [(0, 8), (8, 8), (16, 2)]
RND_ENG = ["A", "A", "D"]
NDVE_C = 25  # corners (of 36) whose blend mults run on DVE; rest on ACT


def cp_counts_upto(gr):
    """(#ACT rounds, #DVE rounds) among global rounds < gr."""
    a = d = 0
    for x in range(gr):
        if RND_ENG[x % 3] == "A":
            a += 1
        else:
            d += 1
    return a, d


def build_graph():
    nc = bacc.Bacc("TRN2")
    # same-engine RAW chains are ordered by the in-order engines; cross-engine
    # hazards are covered by semaphores below.
    nc.detect_race_conditions = False

    x_rows = nc.declare_dram_parameter("x_rows", [NPIX, C], BF16, isOutput=False)
    x_t0 = nc.declare_dram_parameter("x_t0", [128, CT * CW], BF16, isOutput=False)
    x_t1 = nc.declare_dram_parameter("x_t1", [128, CT * CW], BF16, isOutput=False)
    offw = nc.declare_dram_parameter("offw", [128, 18 * 27], BF16, isOutput=False)
    dcnw = nc.declare_dram_parameter("dcnw", [128, 18 * 256], BF16, isOutput=False)
    bnp = nc.declare_dram_parameter("bn", [128, 8], F32, isOutput=False)
    idxb = nc.declare_dram_parameter("idxb", [128, NT * K], F32, isOutput=False)
    ident = nc.declare_dram_parameter("ident", [128, 128], BF16, isOutput=False)
    identf = nc.declare_dram_parameter("identf", [32, 32], F32, isOutput=False)
    out = nc.declare_dram_parameter("out", [2, 128, P], BF16, isOutput=True)

    idx_dram = nc.dram_tensor("idx_dram", [16, NT * 144], I16)

    stack = ExitStack()

    def sb(name, shape, dt):
        return stack.enter_context(nc.sbuf_tensor(name, shape, dt))

    xt_sb = [sb("xt0_sb", [128, CT * CW], BF16), sb("xt1_sb", [128, CT * CW], BF16)]
    offw_sb = sb("offw_sb", [128, 18 * 27], BF16)
    wt_sb = sb("wt_sb", [128, 18 * 256], BF16)
    bn_sb = sb("bn_sb", [128, 8], F32)
    idxb_sb = sb("idxb_sb", [128, NT * K], F32)
    idb_sb = sb("idb_sb", [128, 128], BF16)
    idf_sb = sb("idf_sb", [32, 32], F32)
    off_cm = sb("off_cm", [32, 512], F32)       # [27, 512] used
    off_pix = sb("off_pix", [128, NT * 27], F32)
    m_sb = sb("m_sb", [128, NT * K], F32)
    # field scratch (per-group [128, 36])
    T8 = sb("T8", [128, 36], F32)
    I32A = sb("I32A", [128, 36], I32)
    F8 = sb("F8", [128, 36], F32)
    GT = sb("GT", [128, 36], F32)
    FLY = sb("FLY", [128, 36], F32)
    FLX = sb("FLX", [128, 36], F32)
    FY = sb("FY", [128, 36], F32)
    FX = sb("FX", [128, 36], F32)
    SY8 = sb("SY8", [128, 36], F32)
    SX8 = sb("SX8", [128, 36], F32)
    U1 = sb("U1", [128, 36], F32)
    U0 = sb("U0", [128, 36], F32)
    I0T = sb("I0T", [128, 36], F32)
    s36 = sb("s36", [128, NT * 36], F32)
    idxf = sb("idxf", [128, NT * 18], F32)
    hop1 = sb("hop1", [16, 8 * NT * 18], F32)
    idxs_sb = sb("idxs_sb", [128, NT * 144], I16)
    VBIG = sb("VBIG", [128, 6 * 18 * 512], BF16)  # 6-slot V ring
    S0 = sb("S0", [128, 2304], BF16)
    S1 = sb("S1", [128, 2304], BF16)
    ST4 = [sb("ST4a", [128, 18 * 512], BF16), sb("ST4b", [128, 18 * 512], BF16)]
    out_sb = sb("out_sb", [128, 2 * P], BF16)

    Vb = [VBIG[:, i * 9216 : (i + 1) * 9216] for i in range(6)]
    UNITS = [(t,) for t in range(NT)]

    def unit_of(t):
        return t
    Sb = [S0, S1]
    out_sb_v = out_sb[:].rearrange("p (h n) -> p h n", h=2)

    def st4_dst(tt, c0, nch):
        return ST4[(tt // 4) % 2][:].rearrange("p (c n) -> p c n", n=512)[
            :, c0 : c0 + nch, (tt % 4) * 128 : (tt % 4) * 128 + 128
        ]

    def sem(name):
        return stack.enter_context(nc.semaphore(name))

    d_in = sem("d_in")       # input loads: 10 DMAs x16
    d_h1 = sem("d_h1")       # hop DMAs: 8/group x16
    d_rep = sem("d_rep")     # bounce DMAs: 2/group x16
    d_out = sem("d_out")
    g_sem = sem("g_sem")     # gather completions x16
    p_sem = sem("p_sem")     # gather preps
    pe_conv = sem("pe_conv")  # 1/group
    pe_offt = sem("pe_offt")  # 1/tile
    pe_tr = sem("pe_tr")     # 1/round (3/tile)
    pe_mm = sem("pe_mm")     # 2/group
    a_cm = sem("a_cm")       # 1/group off_cm copy
    a_off = sem("a_off")     # 1/tile off_pix copy
    a_sig = sem("a_sig")     # 1/group sigmoid
    a_mul = sem("a_mul")     # 1/tile ACT blend mults
    a_cp = sem("a_cp")       # ACT ST4 rounds
    a_bn = sem("a_bn")       # 2/group
    v_fld = sem("v_fld")     # 1/group fields (s36+idxf ready)
    v_i16 = sem("v_i16")     # 1/group idx cast
    v_add = sem("v_add")     # 1/tile S ready
    v_cp = sem("v_cp")       # DVE ST4 rounds

    NLOAD = 8

    blk = stack.enter_context(nc.Block())

    with nc.psum_tensor("ps_off0", [32, 512], F32) as ps_off0, nc.psum_tensor(
        "ps_off1", [32, 512], F32
    ) as ps_off1, nc.psum_tensor(
        "ps_tr0", [128, 1024], BF16
    ) as ptr0, nc.psum_tensor(
        "ps_tr1", [128, 1024], BF16
    ) as ptr1, nc.psum_tensor(
        "ps_e0", [128, 1024], F32
    ) as pe0, nc.psum_tensor(
        "ps_e1", [128, 1024], F32
    ) as pe1:
        ps_off = [ps_off0, ps_off1]
        ps_tr = [ptr0, ptr1]
        ps_e = [pe0, pe1]  # [G%2] -> [128, (h, 512)]

        # =================== SYNC (SP): loads, idx plumbing, stores =========
        @blk.sync
        def _(sync):
            # load order matters: PE conv waits d_in>=80 (first 5), DVE fields
            # wait >=96 (through idxb), ACT blends wait >=112 (through bn)
            sync.dma_start(xt_sb[0][:], x_t0[:]).then_inc(d_in, 16)
            sync.dma_start(xt_sb[1][:], x_t1[:]).then_inc(d_in, 16)
            sync.dma_start(offw_sb[:], offw[:]).then_inc(d_in, 16)
            sync.dma_start(idf_sb[:], identf[:]).then_inc(d_in, 16)
            sync.dma_start(idxb_sb[:], idxb[:]).then_inc(d_in, 16)
            sync.dma_start(idb_sb[:], ident[:]).then_inc(d_in, 16)
            sync.dma_start(bn_sb[:], bnp[:]).then_inc(d_in, 16)
            sync.dma_start(wt_sb[:], dcnw[:]).then_inc(d_in, 16)
            for g in range(NG):
                sync.wait_ge(v_fld, g + 1)
                for s in range(8):
                    sync.dma_start(
                        hop1[:, (s * NT * 18) + g * 72 : (s * NT * 18) + (g + 1) * 72],
                        idxf[s * 16 : (s + 1) * 16, g * 72 : (g + 1) * 72],
                    ).then_inc(d_h1, 16)
                sync.wait_ge(v_i16, g + 1)
                sync.dma_start(
                    idx_dram[:, g * 576 : (g + 1) * 576],
                    idxs_sb[0:16, g * 576 : (g + 1) * 576],
                ).then_inc(d_rep, 16)
                sync.wait_ge(d_rep, 32 * g + 16)
                sync.dma_start(
                    idxs_sb[:, g * 576 : (g + 1) * 576],
                    bass.AP(
                        idx_dram,
                        g * 576,
                        [[0, 8], [NT * 144, 16], [1, 576]],
                    ),
                ).then_inc(d_rep, 16)
            for G in range(NG):
                for h in range(2):
                    sync.wait_ge(a_bn, G * 2 + h + 1)
                    sync.dma_start(
                        out[h, :, G * 512 : (G + 1) * 512],
                        out_sb_v[:, h, G * 512 : (G + 1) * 512],
                    ).then_inc(d_out, 16)
            sync.wait_ge(d_out, 16 * 8)

        # =================== GPSIMD: gathers ===================
        @blk.gpsimd
        def _(gp):
            gp.load_library(mlp)

            def prep(u):
                unit = UNITS[u]
                t0, n = unit[0], len(unit)
                gp.wait_ge(d_rep, 32 * (t0 // 4) + 32)
                gp.dma_gather(
                    VBIG[:, (t0 % 6) * 9216 : (t0 % 6 + n) * 9216].rearrange(
                        "p (g c) -> p g c", c=512
                    ),
                    bass.AP(x_rows, 0, [[256, NPIX - 1], [1, 512]]),
                    idxs_sb[:, t0 * 144 : (t0 + n) * 144],
                    n * 18 * 128,
                    n * 18 * 128,
                    512,
                    elem_step=256,
                    single_packet=False,
                    prepare_only=True,
                    sem=g_sem,
                ).then_inc(p_sem, 1)

            prep(0)
            for u in range(len(UNITS)):
                unit = UNITS[u]
                gp.wait_ge(p_sem, u + 1)
                need = unit[0] + len(unit) - 6
                if need > 0:
                    gp.wait_ge(v_add, need)
                gp.trigger_dma(1)
                if u + 1 < len(UNITS):
                    prep(u + 1)

        # =================== PE ===================
        @blk.tensor
        def _(te):
            te.wait_ge(d_in, 4 * 16)
            # offset convs + off transposes per group (transposes land in the
            # not-yet-used ps_e0 bank; einsum G0 starts long after)
            def offtr(g):
                te.wait_ge(a_cm, g + 1)
                for q in range(4):
                    t = g * 4 + q
                    if t >= 2:
                        te.wait_ge(a_off, t - 1)  # ps_e0 slice free
                    te.transpose(
                        ps_e[0][:, (t % 2) * 32 : (t % 2) * 32 + 27].bitcast(F32),
                        off_cm[0:27, q * 128 : (q + 1) * 128],
                        idf_sb[0:27, 0:27],
                    ).then_inc(pe_offt, 1)

            for g in range(NG):
                if g >= 2:
                    te.wait_ge(a_cm, g - 1)  # ps_off[g%2] free
                ins = None
                for ch in range(18):
                    kk, half = ch // 2, ch % 2
                    ky, kx = kk // 3 - 1, kk % 3 - 1
                    rhs = xt_sb[half][:].rearrange("p (r w) -> p r w", w=CW)[
                        :, g * 8 + ky + 1 : g * 8 + ky + 9, kx + 1 : kx + 65
                    ]
                    ins = te.matmul(
                        ps_off[g % 2][0:27, :],
                        offw_sb[:, ch * 27 : (ch + 1) * 27],
                        rhs,
                        start=(ch == 0),
                        stop=(ch == 17),
                        skip_group_check=True,
                    )
                ins.then_inc(pe_conv, 1)
                offtr(g)
            # S transposes + einsum; offtr(3) slots in after tile 0's rounds
            # to break the ACT go3 <-> PE in-order cycle
            te.wait_ge(d_in, 6 * 16)
            for t in range(NT):
                te.wait_ge(v_add, t + 1)
                S = Sb[t % 2]
                for r, (c0, nch) in enumerate(ROUNDS):
                    gr = t * 3 + r
                    if gr >= 2:
                        a_need, d_need = cp_counts_upto(gr - 1)
                        if RND_ENG[(gr - 2) % 3] == "A":
                            te.wait_ge(a_cp, a_need)
                        else:
                            te.wait_ge(v_cp, d_need)
                    bank = ps_tr[gr % 2]
                    ins = None
                    for j in range(nch):
                        c = c0 + j
                        ins = te.transpose(
                            bank[:, j * 128 : (j + 1) * 128],
                            S[:, c * 128 : (c + 1) * 128],
                            idb_sb[:],
                        )
                    ins.then_inc(pe_tr, 1)
                if 4 <= t < 12:
                    # einsum chunk k of group G = t//4 - 1 (9 matmuls),
                    # interleaved with this group's rounds (ST4 dbl-buffered)
                    G = t // 4 - 1
                    k = t % 4
                    if k == 0:
                        a_need, d_need = cp_counts_upto((4 * G + 4) * 3)
                        te.wait_ge(a_cp, a_need)
                        te.wait_ge(v_cp, d_need)
                    h, chalf = k // 2, k % 2
                    ins = None
                    for j in range(9):
                        c = chalf * 9 + j
                        ins = te.matmul(
                            ps_e[G % 2][:, h * 512 : (h + 1) * 512],
                            wt_sb[:, c * 256 + h * 128 : c * 256 + (h + 1) * 128],
                            ST4[G % 2][:, c * 512 : (c + 1) * 512],
                            start=(c == 0),
                            stop=(c == 17),
                            skip_group_check=True,
                        )
                    if k % 2 == 1:
                        ins.then_inc(pe_mm, 1)
                if t == 11:
                    # whole einsum for G2 right after t11 rounds
                    a_need, d_need = cp_counts_upto(12 * 3)
                    te.wait_ge(a_cp, a_need)
                    te.wait_ge(v_cp, d_need)
                    te.wait_ge(a_bn, 2)  # ps_e[0] free (G0 drained)
                    for h in range(2):
                        ins = None
                        for c in range(18):
                            ins = te.matmul(
                                ps_e[0][:, h * 512 : (h + 1) * 512],
                                wt_sb[:, c * 256 + h * 128 : c * 256 + (h + 1) * 128],
                                ST4[0][:, c * 512 : (c + 1) * 512],
                                start=(c == 0),
                                stop=(c == 17),
                                skip_group_check=True,
                            )
                        ins.then_inc(pe_mm, 1)
                if t // 4 == 3:
                    # last group: per-tile einsum chains so the tail only
                    # exposes tile 15's matmuls
                    q = t % 4
                    a_need, d_need = cp_counts_upto((t + 1) * 3)
                    te.wait_ge(a_cp, a_need)
                    te.wait_ge(v_cp, d_need)
                    if q == 0:
                        te.wait_ge(a_bn, 4)  # ps_e[1] free (G1 drained)
                    for h in range(2):
                        ins = None
                        for c in range(18):
                            ins = te.matmul(
                                ps_e[1][:, h * 512 + q * 128 : h * 512 + (q + 1) * 128],
                                wt_sb[:, c * 256 + h * 128 : c * 256 + (h + 1) * 128],
                                ST4[1][:, c * 512 + q * 128 : c * 512 + (q + 1) * 128],
                                start=(c == 0),
                                stop=(c == 17),
                                skip_group_check=True,
                            )
                        if q == 3:
                            ins.then_inc(pe_mm, 1)

        # =================== DVE ===================
        @blk.vector
        def _(v):
            v.wait_ge(d_in, 6 * 16)

            def fields(g):
                # dy/dx/m views for this group's 4 tiles
                dyv = off_pix[:].rearrange("p (t m) -> p t m", m=27)[
                    :, g * 4 : (g + 1) * 4, 0:9
                ]
                dxv = off_pix[:].rearrange("p (t m) -> p t m", m=27)[
                    :, g * 4 : (g + 1) * 4, 9:18
                ]
                mv = m_sb[:, g * 36 : (g + 1) * 36]
                v.wait_ge(a_off, 4 * (g + 1))
                # floor(dy)
                v.tensor_scalar(T8[:], dyv, 8.0, None, ALU.add)
                v.tensor_copy(I32A[:], T8[:])
                v.tensor_copy(F8[:], I32A[:])
                v.tensor_tensor(GT[:], F8[:], T8[:], ALU.is_gt)
                v.tensor_tensor(FLY[:], F8[:], GT[:], ALU.subtract)
                v.scalar_tensor_tensor(FY[:], dyv, 8.0, FLY[:], ALU.add, ALU.subtract)
                v.tensor_scalar(SY8[:], FLY[:], 11.0, 4.0, ALU.min, ALU.max)
                # floor(dx)
                v.tensor_scalar(T8[:], dxv, 8.0, None, ALU.add)
                v.tensor_copy(I32A[:], T8[:])
                v.tensor_copy(F8[:], I32A[:])
                v.tensor_tensor(GT[:], F8[:], T8[:], ALU.is_gt)
                v.tensor_tensor(FLX[:], F8[:], GT[:], ALU.subtract)
                v.scalar_tensor_tensor(FX[:], dxv, 8.0, FLX[:], ALU.add, ALU.subtract)
                v.tensor_scalar(SX8[:], FLX[:], 11.0, 4.0, ALU.min, ALU.max)
                # idx: (SY8*74 + SX8) + IDXB8 ; idx1 = idx0 + 74
                v.scalar_tensor_tensor(I0T[:], SY8[:], 74.0, SX8[:], ALU.mult, ALU.add)
                # idx table group order must match V blend order: g = k*2 + yc
                idxf_v = idxf[:].rearrange("p (t k2 g2) -> p t k2 g2", k2=9, g2=2)
                ib_v = idxb_sb[:].rearrange("p (t k) -> p t k", k=9)[
                    :, g * 4 : (g + 1) * 4, :
                ]
                v.tensor_tensor(
                    idxf_v[:, g * 4 : (g + 1) * 4, :, 0], I0T[:], ib_v, ALU.add
                )
                v.tensor_scalar(
                    idxf_v[:, g * 4 : (g + 1) * 4, :, 1],
                    idxf_v[:, g * 4 : (g + 1) * 4, :, 0],
                    74.0,
                    None,
                    ALU.add,
                )
                # blend scalars: s[(k,yc),xc]; u1 = fy*m, u0 = m-u1
                v.wait_ge(a_sig, g + 1)
                v.tensor_tensor(U1[:], FY[:], mv, ALU.mult)
                v.tensor_tensor(U0[:], mv, U1[:], ALU.subtract)
                s_v = s36[:].rearrange("p (t k yc xc) -> p t k yc xc", k=9, yc=2, xc=2)[
                    :, g * 4 : (g + 1) * 4
                ]
                u0_v = U0[:].rearrange("p (t k) -> p t k", k=9)
                u1_v = U1[:].rearrange("p (t k) -> p t k", k=9)
                fx_v = FX[:].rearrange("p (t k) -> p t k", k=9)
                v.tensor_tensor(s_v[:, :, :, 0, 1], u0_v, fx_v, ALU.mult)
                v.tensor_tensor(s_v[:, :, :, 0, 0], u0_v, s_v[:, :, :, 0, 1], ALU.subtract)
                v.tensor_tensor(s_v[:, :, :, 1, 1], u1_v, fx_v, ALU.mult)
                v.tensor_tensor(
                    s_v[:, :, :, 1, 0], u1_v, s_v[:, :, :, 1, 1], ALU.subtract
                ).then_inc(v_fld, 1)
                # int16 idx wrap (after hop DMAs)
                v.wait_ge(d_h1, 128 * (g + 1))
                v.tensor_copy(
                    idxs_sb[0:16, g * 576 : (g + 1) * 576].rearrange(
                        "q (t g2 s) -> q t g2 s", t=4, g2=18
                    ),
                    hop1[:].rearrange("q (s t g2) -> q t g2 s", s=8, t=NT)[
                        :, g * 4 : (g + 1) * 4
                    ],
                ).then_inc(v_i16, 1)

            def blend(t):
                v.wait_ge(g_sem, 16 * (unit_of(t) + 1))
                V = Vb[t % 6]
                Vv = V.rearrange("p (k yc xc c) -> p k yc xc c", yc=2, xc=2, c=256)
                for ci in range(NDVE_C):
                    g18, xc = ci // 2, ci % 2
                    k, yc = g18 // 2, g18 % 2
                    col = t * 36 + (k * 2 + yc) * 2 + xc
                    v.tensor_scalar(
                        Vv[:, k, yc, xc, :],
                        Vv[:, k, yc, xc, :],
                        s36[:, col : col + 1],
                        None,
                        ALU.mult,
                    )
                v.wait_ge(a_mul, t + 1)
                # H = Vx0 + Vx1 (in place into xc0), S = H(yc0) + H(yc1)
                Vf = V.rearrange("p (g n) -> p g n", n=512)
                v.tensor_tensor(
                    Vf[:, :, 0:256], Vf[:, :, 0:256], Vf[:, :, 256:512], ALU.add
                )
                if t >= 2:
                    v.wait_ge(pe_tr, 3 * (t - 1))  # S[t%2] free
                Vp = V.rearrange("p (k yc n) -> p k yc n", yc=2, n=512)
                S = Sb[t % 2][:].rearrange("p (k c) -> p k c", c=256)
                v.tensor_tensor(
                    S, Vp[:, :, 0, 0:256], Vp[:, :, 1, 0:256], ALU.add
                ).then_inc(v_add, 1)
                # ST4 copy round 2 of tile t-1
                if t >= 1:
                    tt = t - 1
                    gr = tt * 3 + 2
                    v.wait_ge(pe_tr, gr + 1)
                    if tt >= 8:
                        v.wait_ge(pe_mm, 2 * (tt // 4) - 2)
                    c0, nch = ROUNDS[2]
                    v.tensor_copy(
                        st4_dst(tt, c0, nch),
                        ps_tr[gr % 2][:].rearrange("p (c n) -> p c n", n=128)[
                            :, 0:nch, :
                        ],
                    ).then_inc(v_cp, 1)

            fields(0)
            fields(1)
            fields(2)
            fields(3)
            for t in range(NT):
                blend(t)
            # drain tile 15 round 2
            tt = NT - 1
            gr = tt * 3 + 2
            v.wait_ge(pe_tr, gr + 1)
            c0, nch = ROUNDS[2]
            v.tensor_copy(
                st4_dst(tt, c0, nch),
                ps_tr[gr % 2][:].rearrange("p (c n) -> p c n", n=128)[:, 0:nch, :],
            ).then_inc(v_cp, 1)

        # =================== ACT ===================
        @blk.scalar
        def _(a):
            def group_off(g):
                a.wait_ge(pe_conv, g + 1)
                a.copy(off_cm[0:27, :], ps_off[g % 2][0:27, :]).then_inc(a_cm, 1)
                for q in range(4):
                    t = g * 4 + q
                    a.wait_ge(pe_offt, t + 1)
                    a.copy(
                        off_pix[:].rearrange("p (t m) -> p t m", m=27)[:, t, :],
                        ps_e[0][:, (t % 2) * 32 : (t % 2) * 32 + 27].bitcast(F32),
                    ).then_inc(a_off, 1)
                a.activation(
                    m_sb[:, g * 36 : (g + 1) * 36],
                    off_pix[:].rearrange("p (t m) -> p t m", m=27)[
                        :, g * 4 : (g + 1) * 4, 18:27
                    ],
                    ACTF.Sigmoid,
                ).then_inc(a_sig, 1)

            def blend_a(t):
                a.wait_ge(g_sem, 16 * (unit_of(t) + 1))
                a.wait_ge(v_fld, t // 4 + 1)
                V = Vb[t % 6]
                Vv = V.rearrange("p (k yc xc c) -> p k yc xc c", yc=2, xc=2, c=256)
                last = None
                for ci in range(NDVE_C, 36):
                    g18, xc = ci // 2, ci % 2
                    k, yc = g18 // 2, g18 % 2
                    col = t * 36 + (k * 2 + yc) * 2 + xc
                    last = a.mul(
                        Vv[:, k, yc, xc, :],
                        Vv[:, k, yc, xc, :],
                        s36[:, col : col + 1],
                    )
                last.then_inc(a_mul, 1)
                # ST4 copy rounds 0, 1 of tile t-1
                if t >= 1:
                    tt = t - 1
                    for r in (0, 1):
                        gr = tt * 3 + r
                        a.wait_ge(pe_tr, gr + 1)
                        if r == 0 and tt >= 8:
                            a.wait_ge(pe_mm, 2 * (tt // 4) - 2)
                        c0, nch = ROUNDS[r]
                        a.copy(
                            st4_dst(tt, c0, nch),
                            ps_tr[gr % 2][:].rearrange("p (c n) -> p c n", n=128)[
                                :, 0:nch, :
                            ],
                        ).then_inc(a_cp, 1)
                G_bn = {8: 0, 12: 1, 13: 2}.get(t)
                if G_bn is not None:
                    G = G_bn
                    for h in range(2):
                        a.wait_ge(pe_mm, 2 * G + h + 1)
                        a.activation(
                            out_sb_v[:, h, G * 512 : (G + 1) * 512],
                            ps_e[G % 2][:, h * 512 : (h + 1) * 512],
                            ACTF.Relu,
                            bias=bn_sb[:, 2 + h : 3 + h],
                            scale=bn_sb[:, h : h + 1],
                        ).then_inc(a_bn, 1)

            group_off(0)
            group_off(1)
            group_off(2)
            group_off(3)
            for t in range(NT):
                blend_a(t)
            # drain: tile 15 rounds 0, 1, then einsum G3 BN
            tt = NT - 1
            for r in (0, 1):
                gr = tt * 3 + r
                a.wait_ge(pe_tr, gr + 1)
                c0, nch = ROUNDS[r]
                a.copy(
                    st4_dst(tt, c0, nch),
                    ps_tr[gr % 2][:].rearrange("p (c n) -> p c n", n=128)[:, 0:nch, :],
                ).then_inc(a_cp, 1)
            for G in (3,):
                for h in range(2):
                    a.wait_ge(pe_mm, 2 * G + h + 1)
                    a.activation(
                        out_sb_v[:, h, G * 512 : (G + 1) * 512],
                        ps_e[G % 2][:, h * 512 : (h + 1) * 512],
                        ACTF.Relu,
                        bias=bn_sb[:, 2 + h : 3 + h],
                        scale=bn_sb[:, h : h + 1],
                    ).then_inc(a_bn, 1)

    stack.close()
    if not nc.is_finalized():
        nc.finalize()
    return nc


def _host_consts():
    import ml_dtypes

    p = np.arange(128)
    r = p // 64  # row within tile-pair
    c = p % 64
    idxb8 = np.zeros((128, NT, K), np.float32)
    for t in range(NT):
        rho = t * 2 + r  # local output row 0..31
        for k in range(K):
            base = (rho + HALO + KY[k]) * WP + (c + GUARD + KX[k])
            idxb8[:, t, k] = base - (8 * 74 + 8)
    ident = np.eye(128, dtype=ml_dtypes.bfloat16)
    identf = np.eye(32, dtype=np.float32)
    return idxb8.reshape(128, NT * K), np.asarray(ident), identf


def make_in_maps(x, offset_w, dcn_w, gamma, beta, moving_mean, moving_var):
    import ml_dtypes

    x = np.ascontiguousarray(x, np.float32)
    idxb8, ident, identf = _host_consts()

    # offw [128, 18*27]: row (kk*256+cin) -> [cin%128, (kk*2+cin//128)*27+m]
    ow = np.asarray(offset_w, np.float32).reshape(18, 128, 27)
    offw_h = np.ascontiguousarray(
        np.transpose(ow, (1, 0, 2)).reshape(128, 18 * 27).astype(ml_dtypes.bfloat16)
    )
    dw = np.asarray(dcn_w, np.float32).reshape(18, 128, 256)
    dcnw_h = np.ascontiguousarray(
        np.transpose(dw, (1, 0, 2)).reshape(128, 18 * 256).astype(ml_dtypes.bfloat16)
    )

    inv_f = np.asarray(gamma, np.float32) / np.sqrt(
        np.asarray(moving_var, np.float32) + BN_EPS
    )
    ab_f = np.asarray(beta, np.float32) - np.asarray(moving_mean, np.float32) * inv_f
    bn_h = np.zeros((128, 8), np.float32)
    for h in range(2):
        bn_h[:, h] = inv_f.reshape(2, 128)[h]
        bn_h[:, 2 + h] = ab_f.reshape(2, 128)[h]

    in_maps = []
    for core in range(NCORES):
        r0 = core * RPC
        b = r0 // H
        rb = r0 % H
        # padded rows rb-HALO .. rb+RPC+HALO+1 (for conv we need rb-1..rb+32)
        pad = np.zeros((RIN, WP, C), np.float32)
        lo = rb - HALO
        hi = rb + RPC + HALO
        slo, shi = max(lo, 0), min(hi, H)
        pad[slo - lo : shi - lo, GUARD : GUARD + W] = x[b, slo:shi]
        x_rows_h = np.ascontiguousarray(
            pad.reshape(RIN * WP, C).astype(ml_dtypes.bfloat16)
        )
        # conv x_T: rows rb-1 .. rb+32 (34), cols -1..64 (66), ch-on-partition
        conv_rows = pad[HALO - 1 : HALO - 1 + CT, GUARD - 1 : GUARD - 1 + CW]
        x_t = np.transpose(conv_rows, (2, 0, 1)).reshape(C, CT * CW)
        x_t16 = x_t.astype(ml_dtypes.bfloat16)
        in_maps.append(
            dict(
                x_rows=x_rows_h,
                x_t0=np.ascontiguousarray(x_t16[0:128]),
                x_t1=np.ascontiguousarray(x_t16[128:256]),
                offw=offw_h,
                dcnw=dcnw_h,
                bn=bn_h,
                idxb=idxb8,
                ident=ident,
                identf=identf,
            )
        )
    return in_maps


def kernel(x, offset_w, dcn_w, gamma, beta, moving_mean, moving_var):
    in_maps = make_in_maps(
        x, offset_w, dcn_w, gamma, beta, moving_mean, moving_var
    )
    nc = build_graph()
    res = run_bass_kernel_spmd(nc, in_maps, list(range(NCORES)))
    outs = res.results if hasattr(res, "results") else res

    full = np.zeros((B, H, W, F), np.float32)
    for core in range(NCORES):
        o = np.asarray(outs[core]["out"]).astype(np.float32)  # [2, 128, P]
        o = o.reshape(256, P).T.reshape(RPC, W, F)
        r0 = core * RPC
        full[r0 // H, r0 % H : r0 % H + RPC] = o
    return full


if __name__ == "__main__":
    import reference

    inp = {k: np.asarray(v) for k, v in reference.setup_inputs().items()}
    got = kernel(**inp)
    print("kernel ran, shape", got.shape)


# revision 59
# speedup vs baseline: 1.0703x; 1.0339x over previous
"""DCNv2 (offset conv -> bilinear-sampled modulated deform conv) + BN + ReLU
on 8 TRN2 NeuronCores — v2 pipelined.

Per core (data-parallel over the 256 global rows, 32 rows/core):
  - host precomputes x_rows [42x74 pixel-rows, 256ch] bf16 (5-guard-col /
    5-halo-row padded) so the gather sources the DRAM input directly, plus
    the channel-on-partition x_T for the offset conv and bf16 weights.
  - per 4-tile group: offset conv on PE (channel-major [27,512] psum) ->
    per-tile PE transpose -> slim f32 field ops on floor(dy)/floor(dx)
    directly (clip to +-4/+3; guards absorb all out-of-image taps) ->
    int16 idx wrap + DRAM-bounce replication -> gpsimd dma_gather of
    (x0,x1) pairs (1024B descriptors), 18 groups per pixel tile.
  - blend: 36 per-corner tensor_scalar mults (24 DVE / 12 ACT, 4x bf16),
    pair-adds on DVE; PE transposes S chunks into two [128,1024] bf16 psum
    banks (3 rounds), copies to ST4 (DVE/ACT), einsum accumulates in PSUM,
    BN+ReLU fused in the ACT drain, bf16 stores (host casts to f32).
"""

import sys

import numpy as np

sys.path.insert(0, "/opt/trn_rl_repo")

import concourse.bacc as bacc
import concourse.bass as bass
import concourse.mybir as mybir
from concourse.bass_utils import run_bass_kernel_spmd
from concourse.library_config import mlp
from contextlib import ExitStack

F32 = mybir.dt.float32
BF16 = mybir.dt.bfloat16
I16 = mybir.dt.int16
I32 = mybir.dt.int32
ALU = mybir.AluOpType
ACTF = mybir.ActivationFunctionType

B, H, W, C, F = 4, 64, 64, 256, 256
K = 9
NCORES = 8
RPC = (B * H) // NCORES      # 32 output rows per core
P = RPC * W                  # 2048 pixels per core
NT = P // 128                # 16 pixel tiles
NG = 4                       # tile groups (4 tiles = 8 rows each)
HALO = 5                     # rows of halo each side
RIN = RPC + 2 * HALO         # 42 stored rows
GUARD = 5                    # zero guard cols each side
WP = W + 2 * GUARD           # 74 stored cols
NPIX = RIN * WP              # 3108 x_rows pixel-rows
CT = 34                      # conv x_T rows (-1 .. 32)
CW = 66                      # conv x_T cols (-1 .. 64)
BN_EPS = 1e-3

KY = np.array([-1, -1, -1, 0, 0, 0, 1, 1, 1], np.float32)
KX = np.array([-1, 0, 1, -1, 0, 1, -1, 0, 1], np.float32)

# ST4 transpose copy rounds: (first chunk, n chunks); engines A, A, D
ROUNDS = [(0, 8), (8, 8), (16, 2)]
RND_ENG = ["A", "A", "D"]
NDVE_C = 25  # corners (of 36) whose blend mults run on DVE; rest on ACT


def cp_counts_upto(gr):
    """(#ACT rounds, #DVE rounds) among global rounds < gr."""
    a = d = 0
    for x in range(gr):
        if RND_ENG[x % 3] == "A":
            a += 1
        else:
            d += 1
    return a, d


def build_graph():
    nc = bacc.Bacc("TRN2")
    # same-engine RAW chains are ordered by the in-order engines; cross-engine
    # hazards are covered by semaphores below.
    nc.detect_race_conditions = False

    x_rows = nc.declare_dram_parameter("x_rows", [NPIX, C], BF16, isOutput=False)
    x_t0 = nc.declare_dram_parameter("x_t0", [128, CT * CW], BF16, isOutput=False)
    x_t1 = nc.declare_dram_parameter("x_t1", [128, CT * CW], BF16, isOutput=False)
    offw = nc.declare_dram_parameter("offw", [128, 18 * 27], BF16, isOutput=False)
    dcnw = nc.declare_dram_parameter("dcnw", [128, 18 * 256], BF16, isOutput=False)
    bnp = nc.declare_dram_parameter("bn", [128, 8], F32, isOutput=False)
    idxb = nc.declare_dram_parameter("idxb", [128, NT * K], F32, isOutput=False)
    ident = nc.declare_dram_parameter("ident", [128, 128], BF16, isOutput=False)
    identf = nc.declare_dram_parameter("identf", [32, 32], F32, isOutput=False)
    out = nc.declare_dram_parameter("out", [2, 128, P], BF16, isOutput=True)

    idx_dram = nc.dram_tensor("idx_dram", [16, NT * 144], I16)

    stack = ExitStack()

    def sb(name, shape, dt):
        return stack.enter_context(nc.sbuf_tensor(name, shape, dt))

    xt_sb = [sb("xt0_sb", [128, CT * CW], BF16), sb("xt1_sb", [128, CT * CW], BF16)]
    offw_sb = sb("offw_sb", [128, 18 * 27], BF16)
    wt_sb = sb("wt_sb", [128, 18 * 256], BF16)
    bn_sb = sb("bn_sb", [128, 8], F32)
    idxb_sb = sb("idxb_sb", [128, NT * K], F32)
    idb_sb = sb("idb_sb", [128, 128], BF16)
    idf_sb = sb("idf_sb", [32, 32], F32)
    off_cm = sb("off_cm", [32, 512], F32)       # [27, 512] used
    off_pix = sb("off_pix", [128, NT * 27], F32)
    m_sb = sb("m_sb", [128, NT * K], F32)
    # field scratch (per-group [128, 36])
    T8 = sb("T8", [128, 36], F32)
    I32A = sb("I32A", [128, 36], I32)
    F8 = sb("F8", [128, 36], F32)
    GT = sb("GT", [128, 36], F32)
    FLY = sb("FLY", [128, 36], F32)
    FLX = sb("FLX", [128, 36], F32)
    FY = sb("FY", [128, 36], F32)
    FX = sb("FX", [128, 36], F32)
    SY8 = sb("SY8", [128, 36], F32)
    SX8 = sb("SX8", [128, 36], F32)
    U1 = sb("U1", [128, 36], F32)
    U0 = sb("U0", [128, 36], F32)
    I0T = sb("I0T", [128, 36], F32)
    s36 = sb("s36", [128, NT * 36], F32)
    idxf = sb("idxf", [128, NT * 18], F32)
    hop1 = sb("hop1", [16, 8 * NT * 18], F32)
    idxs_sb = sb("idxs_sb", [128, NT * 144], I16)
    VBIG = sb("VBIG", [128, 6 * 18 * 512], BF16)  # 6-slot V ring
    S0 = sb("S0", [128, 2304], BF16)
    S1 = sb("S1", [128, 2304], BF16)
    ST4 = [sb("ST4a", [128, 18 * 512], BF16), sb("ST4b", [128, 18 * 512], BF16)]
    out_sb = sb("out_sb", [128, 2 * P], BF16)

    Vb = [VBIG[:, i * 9216 : (i + 1) * 9216] for i in range(6)]
    UNITS = [(t,) for t in range(NT)]

    def unit_of(t):
        return t
    Sb = [S0, S1]
    out_sb_v = out_sb[:].rearrange("p (h n) -> p h n", h=2)

    def st4_dst(tt, c0, nch):
        return ST4[(tt // 4) % 2][:].rearrange("p (c n) -> p c n", n=512)[
            :, c0 : c0 + nch, (tt % 4) * 128 : (tt % 4) * 128 + 128
        ]

    def sem(name):
        return stack.enter_context(nc.semaphore(name))

    d_in = sem("d_in")       # input loads: 10 DMAs x16
    d_h1 = sem("d_h1")       # hop DMAs: 8/group x16
    d_rep = sem("d_rep")     # bounce DMAs: 2/group x16
    d_out = sem("d_out")
    g_sem = sem("g_sem")     # gather completions x16
    p_sem = sem("p_sem")     # gather preps
    pe_conv = sem("pe_conv")  # 1/group
    pe_offt = sem("pe_offt")  # 1/tile
    pe_tr = sem("pe_tr")     # 1/round (3/tile)
    pe_mm = sem("pe_mm")     # 2/group
    a_cm = sem("a_cm")       # 1/group off_cm copy
    a_off = sem("a_off")     # 1/tile off_pix copy
    a_sig = sem("a_sig")     # 1/group sigmoid
    a_mul = sem("a_mul")     # 1/tile ACT blend mults
    a_cp = sem("a_cp")       # ACT ST4 rounds
    a_bn = sem("a_bn")       # 2/group
    v_fld = sem("v_fld")     # 1/group fields (s36+idxf ready)
    v_i16 = sem("v_i16")     # 1/group idx cast
    v_add = sem("v_add")     # 1/tile S ready
    v_cp = sem("v_cp")       # DVE ST4 rounds

    NLOAD = 8

    blk = stack.enter_context(nc.Block())

    with nc.psum_tensor("ps_off0", [32, 512], F32) as ps_off0, nc.psum_tensor(
        "ps_off1", [32, 512], F32
    ) as ps_off1, nc.psum_tensor(
        "ps_tr0", [128, 1024], BF16
    ) as ptr0, nc.psum_tensor(
        "ps_tr1", [128, 1024], BF16
    ) as ptr1, nc.psum_tensor(
        "ps_e0", [128, 1024], F32
    ) as pe0, nc.psum_tensor(
        "ps_e1", [128, 1024], F32
    ) as pe1:
        ps_off = [ps_off0, ps_off1]
        ps_tr = [ptr0, ptr1]
        ps_e = [pe0, pe1]  # [G%2] -> [128, (h, 512)]

        # =================== SYNC (SP): loads, idx plumbing, stores =========
        @blk.sync
        def _(sync):
            # load order matters: PE conv waits d_in>=80 (first 5), DVE fields
            # wait >=96 (through idxb), ACT blends wait >=112 (through bn)
            sync.dma_start(xt_sb[0][:], x_t0[:]).then_inc(d_in, 16)
            sync.dma_start(xt_sb[1][:], x_t1[:]).then_inc(d_in, 16)
            sync.dma_start(offw_sb[:], offw[:]).then_inc(d_in, 16)
            sync.dma_start(idf_sb[:], identf[:]).then_inc(d_in, 16)
            sync.dma_start(idxb_sb[:], idxb[:]).then_inc(d_in, 16)
            sync.dma_start(idb_sb[:], ident[:]).then_inc(d_in, 16)
            sync.dma_start(bn_sb[:], bnp[:]).then_inc(d_in, 16)
            sync.dma_start(wt_sb[:], dcnw[:]).then_inc(d_in, 16)
            for g in range(NG):
                sync.wait_ge(v_fld, g + 1)
                for s in range(4 if g == 0 else 8):
                    sync.dma_start(
                        hop1[:, (s * NT * 18) + g * 72 : (s * NT * 18) + (g + 1) * 72],
                        idxf[s * 16 : (s + 1) * 16, g * 72 : (g + 1) * 72],
                    ).then_inc(d_h1, 16)
                sync.wait_ge(v_i16, g + 1)
                sync.dma_start(
                    idx_dram[:, g * 576 : (g + 1) * 576],
                    idxs_sb[0:16, g * 576 : (g + 1) * 576],
                ).then_inc(d_rep, 16)
                sync.wait_ge(d_rep, 32 * g + 16)
                sync.dma_start(
                    idxs_sb[:, g * 576 : (g + 1) * 576],
                    bass.AP(
                        idx_dram,
                        g * 576,
                        [[0, 8], [NT * 144, 16], [1, 576]],
                    ),
                ).then_inc(d_rep, 16)
            for G in range(NG):
                for h in range(2):
                    sync.wait_ge(a_bn, G * 2 + h + 1)
                    sync.dma_start(
                        out[h, :, G * 512 : (G + 1) * 512],
                        out_sb_v[:, h, G * 512 : (G + 1) * 512],
                    ).then_inc(d_out, 16)
            sync.wait_ge(d_out, 16 * 8)

        # =================== GPSIMD: gathers ===================
        @blk.gpsimd
        def _(gp):
            gp.load_library(mlp)

            def prep(u):
                unit = UNITS[u]
                t0, n = unit[0], len(unit)
                gp.wait_ge(d_rep, 32 * (t0 // 4) + 32)
                gp.dma_gather(
                    VBIG[:, (t0 % 6) * 9216 : (t0 % 6 + n) * 9216].rearrange(
                        "p (g c) -> p g c", c=512
                    ),
                    bass.AP(x_rows, 0, [[256, NPIX - 1], [1, 512]]),
                    idxs_sb[:, t0 * 144 : (t0 + n) * 144],
                    n * 18 * 128,
                    n * 18 * 128,
                    512,
                    elem_step=256,
                    single_packet=False,
                    prepare_only=True,
                    sem=g_sem,
                ).then_inc(p_sem, 1)

            prep(0)
            for u in range(len(UNITS)):
                unit = UNITS[u]
                gp.wait_ge(p_sem, u + 1)
                need = unit[0] + len(unit) - 6
                if need > 0:
                    gp.wait_ge(v_add, need)
                gp.trigger_dma(1)
                if u + 1 < len(UNITS):
                    prep(u + 1)

        # =================== PE ===================
        @blk.tensor
        def _(te):
            te.wait_ge(d_in, 4 * 16)
            # offset convs + off transposes per group (transposes land in the
            # not-yet-used ps_e0 bank; einsum G0 starts long after)
            def offtr(g):
                te.wait_ge(a_cm, g + 1)
                for q in range(4):
                    t = g * 4 + q
                    if t >= 2:
                        te.wait_ge(a_off, t - 1)  # ps_e0 slice free
                    te.transpose(
                        ps_e[0][:, (t % 2) * 32 : (t % 2) * 32 + 27].bitcast(F32),
                        off_cm[0:27, q * 128 : (q + 1) * 128],
                        idf_sb[0:27, 0:27],
                    ).then_inc(pe_offt, 1)

            for g in range(NG):
                if g >= 2:
                    te.wait_ge(a_cm, g - 1)  # ps_off[g%2] free
                ins = None
                for ch in range(18):
                    kk, half = ch // 2, ch % 2
                    ky, kx = kk // 3 - 1, kk % 3 - 1
                    rhs = xt_sb[half][:].rearrange("p (r w) -> p r w", w=CW)[
                        :, g * 8 + ky + 1 : g * 8 + ky + 9, kx + 1 : kx + 65
                    ]
                    ins = te.matmul(
                        ps_off[g % 2][0:27, :],
                        offw_sb[:, ch * 27 : (ch + 1) * 27],
                        rhs,
                        start=(ch == 0),
                        stop=(ch == 17),
                        skip_group_check=True,
                    )
                ins.then_inc(pe_conv, 1)
                offtr(g)
            # S transposes + einsum; offtr(3) slots in after tile 0's rounds
            # to break the ACT go3 <-> PE in-order cycle
            te.wait_ge(d_in, 6 * 16)
            for t in range(NT):
                te.wait_ge(v_add, t + 1)
                S = Sb[t % 2]
                for r, (c0, nch) in enumerate(ROUNDS):
                    gr = t * 3 + r
                    if gr >= 2:
                        a_need, d_need = cp_counts_upto(gr - 1)
                        if RND_ENG[(gr - 2) % 3] == "A":
                            te.wait_ge(a_cp, a_need)
                        else:
                            te.wait_ge(v_cp, d_need)
                    bank = ps_tr[gr % 2]
                    ins = None
                    for j in range(nch):
                        c = c0 + j
                        ins = te.transpose(
                            bank[:, j * 128 : (j + 1) * 128],
                            S[:, c * 128 : (c + 1) * 128],
                            idb_sb[:],
                        )
                    ins.then_inc(pe_tr, 1)
                if 4 <= t:
                    # einsum chunk k of group G = t//4 - 1 (9 matmuls),
                    # interleaved with this group's rounds (ST4 dbl-buffered)
                    G = t // 4 - 1
                    k = t % 4
                    if k == 0:
                        a_need, d_need = cp_counts_upto((4 * G + 4) * 3)
                        te.wait_ge(a_cp, a_need)
                        te.wait_ge(v_cp, d_need)
                        if G >= 2:
                            te.wait_ge(a_bn, 2 * (G - 1))
                    h, chalf = k // 2, k % 2
                    ins = None
                    for j in range(9):
                        c = chalf * 9 + j
                        ins = te.matmul(
                            ps_e[G % 2][:, h * 512 : (h + 1) * 512],
                            wt_sb[:, c * 256 + h * 128 : c * 256 + (h + 1) * 128],
                            ST4[G % 2][:, c * 512 : (c + 1) * 512],
                            start=(c == 0),
                            stop=(c == 17),
                            skip_group_check=True,
                        )
                    if k % 2 == 1:
                        ins.then_inc(pe_mm, 1)
                if t // 4 == 3:
                    # last group: per-tile einsum chains so the tail only
                    # exposes tile 15's matmuls
                    q = t % 4
                    a_need, d_need = cp_counts_upto((t + 1) * 3)
                    te.wait_ge(a_cp, a_need)
                    te.wait_ge(v_cp, d_need)
                    if q == 0:
                        te.wait_ge(a_bn, 4)  # ps_e[1] free (G1 drained)
                    for h in range(2):
                        ins = None
                        for c in range(18):
                            ins = te.matmul(
                                ps_e[1][:, h * 512 + q * 128 : h * 512 + (q + 1) * 128],
                                wt_sb[:, c * 256 + h * 128 : c * 256 + (h + 1) * 128],
                                ST4[1][:, c * 512 + q * 128 : c * 512 + (q + 1) * 128],
                                start=(c == 0),
                                stop=(c == 17),
                                skip_group_check=True,
                            )
                        if q == 3:
                            ins.then_inc(pe_mm, 1)
            # G2 einsum drains after the tile loop (chunks t16..t19 don't
            # exist; emit its remaining work here)
            # (handled above: G2 chunks ran at t12..15 via the 4<=t branch)

        # =================== DVE ===================
        @blk.vector
        def _(v):
            v.wait_ge(d_in, 6 * 16)

            def fields(g):
                # dy/dx/m views for this group's 4 tiles
                dyv = off_pix[:].rearrange("p (t m) -> p t m", m=27)[
                    :, g * 4 : (g + 1) * 4, 0:9
                ]
                dxv = off_pix[:].rearrange("p (t m) -> p t m", m=27)[
                    :, g * 4 : (g + 1) * 4, 9:18
                ]
                mv = m_sb[:, g * 36 : (g + 1) * 36]
                v.wait_ge(a_off, 4 * (g + 1))
                # floor(dy)
                v.tensor_scalar(T8[:], dyv, 8.0, None, ALU.add)
                v.tensor_copy(I32A[:], T8[:])
                v.tensor_copy(F8[:], I32A[:])
                v.tensor_tensor(GT[:], F8[:], T8[:], ALU.is_gt)
                v.tensor_tensor(FLY[:], F8[:], GT[:], ALU.subtract)
                v.scalar_tensor_tensor(FY[:], dyv, 8.0, FLY[:], ALU.add, ALU.subtract)
                v.tensor_scalar(SY8[:], FLY[:], 11.0, 4.0, ALU.min, ALU.max)
                # floor(dx)
                v.tensor_scalar(T8[:], dxv, 8.0, None, ALU.add)
                v.tensor_copy(I32A[:], T8[:])
                v.tensor_copy(F8[:], I32A[:])
                v.tensor_tensor(GT[:], F8[:], T8[:], ALU.is_gt)
                v.tensor_tensor(FLX[:], F8[:], GT[:], ALU.subtract)
                v.scalar_tensor_tensor(FX[:], dxv, 8.0, FLX[:], ALU.add, ALU.subtract)
                v.tensor_scalar(SX8[:], FLX[:], 11.0, 4.0, ALU.min, ALU.max)
                # idx: (SY8*74 + SX8) + IDXB8 ; idx1 = idx0 + 74
                v.scalar_tensor_tensor(I0T[:], SY8[:], 74.0, SX8[:], ALU.mult, ALU.add)
                # idx table group order must match V blend order: g = k*2 + yc
                idxf_v = idxf[:].rearrange("p (t k2 g2) -> p t k2 g2", k2=9, g2=2)
                ib_v = idxb_sb[:].rearrange("p (t k) -> p t k", k=9)[
                    :, g * 4 : (g + 1) * 4, :
                ]
                v.tensor_tensor(
                    idxf_v[:, g * 4 : (g + 1) * 4, :, 0], I0T[:], ib_v, ALU.add
                )
                v.tensor_scalar(
                    idxf_v[:, g * 4 : (g + 1) * 4, :, 1],
                    idxf_v[:, g * 4 : (g + 1) * 4, :, 0],
                    74.0,
                    None,
                    ALU.add,
                )
                # blend scalars: s[(k,yc),xc]; u1 = fy*m, u0 = m-u1
                v.wait_ge(a_sig, g + 1)
                v.tensor_tensor(U1[:], FY[:], mv, ALU.mult)
                v.tensor_tensor(U0[:], mv, U1[:], ALU.subtract)
                s_v = s36[:].rearrange("p (t k yc xc) -> p t k yc xc", k=9, yc=2, xc=2)[
                    :, g * 4 : (g + 1) * 4
                ]
                u0_v = U0[:].rearrange("p (t k) -> p t k", k=9)
                u1_v = U1[:].rearrange("p (t k) -> p t k", k=9)
                fx_v = FX[:].rearrange("p (t k) -> p t k", k=9)
                v.tensor_tensor(s_v[:, :, :, 0, 1], u0_v, fx_v, ALU.mult)
                v.tensor_tensor(s_v[:, :, :, 0, 0], u0_v, s_v[:, :, :, 0, 1], ALU.subtract)
                v.tensor_tensor(s_v[:, :, :, 1, 1], u1_v, fx_v, ALU.mult)
                v.tensor_tensor(
                    s_v[:, :, :, 1, 0], u1_v, s_v[:, :, :, 1, 1], ALU.subtract
                ).then_inc(v_fld, 1)
                # int16 idx wrap (after hop DMAs)
                v.wait_ge(d_h1, 128 * (g + 1))
                v.tensor_copy(
                    idxs_sb[0:16, g * 576 : (g + 1) * 576].rearrange(
                        "q (t g2 s) -> q t g2 s", t=4, g2=18
                    ),
                    hop1[:].rearrange("q (s t g2) -> q t g2 s", s=8, t=NT)[
                        :, g * 4 : (g + 1) * 4
                    ],
                ).then_inc(v_i16, 1)

            def blend(t):
                v.wait_ge(g_sem, 16 * (unit_of(t) + 1))
                V = Vb[t % 6]
                Vv = V.rearrange("p (k yc xc c) -> p k yc xc c", yc=2, xc=2, c=256)
                for ci in range(NDVE_C):
                    g18, xc = ci // 2, ci % 2
                    k, yc = g18 // 2, g18 % 2
                    col = t * 36 + (k * 2 + yc) * 2 + xc
                    v.tensor_scalar(
                        Vv[:, k, yc, xc, :],
                        Vv[:, k, yc, xc, :],
                        s36[:, col : col + 1],
                        None,
                        ALU.mult,
                    )
                v.wait_ge(a_mul, t + 1)
                # H = Vx0 + Vx1 (in place into xc0), S = H(yc0) + H(yc1)
                Vf = V.rearrange("p (g n) -> p g n", n=512)
                v.tensor_tensor(
                    Vf[:, :, 0:256], Vf[:, :, 0:256], Vf[:, :, 256:512], ALU.add
                )
                if t >= 2:
                    v.wait_ge(pe_tr, 3 * (t - 1))  # S[t%2] free
                Vp = V.rearrange("p (k yc n) -> p k yc n", yc=2, n=512)
                S = Sb[t % 2][:].rearrange("p (k c) -> p k c", c=256)
                v.tensor_tensor(
                    S, Vp[:, :, 0, 0:256], Vp[:, :, 1, 0:256], ALU.add
                ).then_inc(v_add, 1)
                # ST4 copy round 2 of tile t-1
                if t >= 1:
                    tt = t - 1
                    gr = tt * 3 + 2
                    v.wait_ge(pe_tr, gr + 1)
                    if tt >= 8:
                        v.wait_ge(pe_mm, 2 * (tt // 4) - 2)
                    c0, nch = ROUNDS[2]
                    v.tensor_copy(
                        st4_dst(tt, c0, nch),
                        ps_tr[gr % 2][:].rearrange("p (c n) -> p c n", n=128)[
                            :, 0:nch, :
                        ],
                    ).then_inc(v_cp, 1)

            fields(0)
            fields(1)
            fields(2)
            fields(3)
            for t in range(NT):
                blend(t)
            # drain tile 15 round 2
            tt = NT - 1
            gr = tt * 3 + 2
            v.wait_ge(pe_tr, gr + 1)
            c0, nch = ROUNDS[2]
            v.tensor_copy(
                st4_dst(tt, c0, nch),
                ps_tr[gr % 2][:].rearrange("p (c n) -> p c n", n=128)[:, 0:nch, :],
            ).then_inc(v_cp, 1)

        # =================== ACT ===================
        @blk.scalar
        def _(a):
            def group_off(g):
                a.wait_ge(pe_conv, g + 1)
                a.copy(off_cm[0:27, :], ps_off[g % 2][0:27, :]).then_inc(a_cm, 1)
                for q in range(4):
                    t = g * 4 + q
                    a.wait_ge(pe_offt, t + 1)
                    a.copy(
                        off_pix[:].rearrange("p (t m) -> p t m", m=27)[:, t, :],
                        ps_e[0][:, (t % 2) * 32 : (t % 2) * 32 + 27].bitcast(F32),
                    ).then_inc(a_off, 1)
                a.activation(
                    m_sb[:, g * 36 : (g + 1) * 36],
                    off_pix[:].rearrange("p (t m) -> p t m", m=27)[
                        :, g * 4 : (g + 1) * 4, 18:27
                    ],
                    ACTF.Sigmoid,
                ).then_inc(a_sig, 1)

            def blend_a(t):
                a.wait_ge(g_sem, 16 * (unit_of(t) + 1))
                a.wait_ge(v_fld, t // 4 + 1)
                V = Vb[t % 6]
                Vv = V.rearrange("p (k yc xc c) -> p k yc xc c", yc=2, xc=2, c=256)
                last = None
                for ci in range(NDVE_C, 36):
                    g18, xc = ci // 2, ci % 2
                    k, yc = g18 // 2, g18 % 2
                    col = t * 36 + (k * 2 + yc) * 2 + xc
                    last = a.mul(
                        Vv[:, k, yc, xc, :],
                        Vv[:, k, yc, xc, :],
                        s36[:, col : col + 1],
                    )
                last.then_inc(a_mul, 1)
                # ST4 copy rounds 0, 1 of tile t-1
                if t >= 1:
                    tt = t - 1
                    for r in (0, 1):
                        gr = tt * 3 + r
                        a.wait_ge(pe_tr, gr + 1)
                        if r == 0 and tt >= 8:
                            a.wait_ge(pe_mm, 2 * (tt // 4) - 2)
                        c0, nch = ROUNDS[r]
                        a.copy(
                            st4_dst(tt, c0, nch),
                            ps_tr[gr % 2][:].rearrange("p (c n) -> p c n", n=128)[
                                :, 0:nch, :
                            ],
                        ).then_inc(a_cp, 1)
                G_bn = {8: 0, 12: 1}.get(t)
                if G_bn is not None:
                    G = G_bn
                    for h in range(2):
                        a.wait_ge(pe_mm, 2 * G + h + 1)
                        a.activation(
                            out_sb_v[:, h, G * 512 : (G + 1) * 512],
                            ps_e[G % 2][:, h * 512 : (h + 1) * 512],
                            ACTF.Relu,
                            bias=bn_sb[:, 2 + h : 3 + h],
                            scale=bn_sb[:, h : h + 1],
                        ).then_inc(a_bn, 1)

            group_off(0)
            # group 0's other 4 idx hop DMAs run here so the first gather's
            # idx chain doesn't serialize behind all 8 on SP
            a.wait_ge(v_fld, 1)
            for s in range(4, 8):
                a.dma_start(
                    hop1[:, (s * NT * 18) + 0 : (s * NT * 18) + 72],
                    idxf[s * 16 : (s + 1) * 16, 0:72],
                ).then_inc(d_h1, 16)
            group_off(1)
            group_off(2)
            group_off(3)
            for t in range(NT):
                blend_a(t)
            # drain: tile 15 rounds 0, 1, then einsum G3 BN
            tt = NT - 1
            for r in (0, 1):
                gr = tt * 3 + r
                a.wait_ge(pe_tr, gr + 1)
                c0, nch = ROUNDS[r]
                a.copy(
                    st4_dst(tt, c0, nch),
                    ps_tr[gr % 2][:].rearrange("p (c n) -> p c n", n=128)[:, 0:nch, :],
                ).then_inc(a_cp, 1)
            for G in (2, 3):
                for h in range(2):
                    a.wait_ge(pe_mm, 2 * G + h + 1)
                    a.activation(
                        out_sb_v[:, h, G * 512 : (G + 1) * 512],
                        ps_e[G % 2][:, h * 512 : (h + 1) * 512],
                        ACTF.Relu,
                        bias=bn_sb[:, 2 + h : 3 + h],
                        scale=bn_sb[:, h : h + 1],
                    ).then_inc(a_bn, 1)

    stack.close()
    if not nc.is_finalized():
        nc.finalize()
    return nc


def _host_consts():
    import ml_dtypes

    p = np.arange(128)
    r = p // 64  # row within tile-pair
    c = p % 64
    idxb8 = np.zeros((128, NT, K), np.float32)
    for t in range(NT):
        rho = t * 2 + r  # local output row 0..31
        for k in range(K):
            base = (rho + HALO + KY[k]) * WP + (c + GUARD + KX[k])
            idxb8[:, t, k] = base - (8 * 74 + 8)
    ident = np.eye(128, dtype=ml_dtypes.bfloat16)
    identf = np.eye(32, dtype=np.float32)
    return idxb8.reshape(128, NT * K), np.asarray(ident), identf


def make_in_maps(x, offset_w, dcn_w, gamma, beta, moving_mean, moving_var):
    import ml_dtypes

    x = np.ascontiguousarray(x, np.float32)
    idxb8, ident, identf = _host_consts()

    # offw [128, 18*27]: row (kk*256+cin) -> [cin%128, (kk*2+cin//128)*27+m]
    ow = np.asarray(offset_w, np.float32).reshape(18, 128, 27)
    offw_h = np.ascontiguousarray(
        np.transpose(ow, (1, 0, 2)).reshape(128, 18 * 27).astype(ml_dtypes.bfloat16)
    )
    dw = np.asarray(dcn_w, np.float32).reshape(18, 128, 256)
    dcnw_h = np.ascontiguousarray(
        np.transpose(dw, (1, 0, 2)).reshape(128, 18 * 256).astype(ml_dtypes.bfloat16)
    )

    inv_f = np.asarray(gamma, np.float32) / np.sqrt(
        np.asarray(moving_var, np.float32) + BN_EPS
    )
    ab_f = np.asarray(beta, np.float32) - np.asarray(moving_mean, np.float32) * inv_f
    bn_h = np.zeros((128, 8), np.float32)
    for h in range(2):
        bn_h[:, h] = inv_f.reshape(2, 128)[h]
        bn_h[:, 2 + h] = ab_f.reshape(2, 128)[h]

    in_maps = []
    for core in range(NCORES):
        r0 = core * RPC
        b = r0 // H
        rb = r0 % H
        # padded rows rb-HALO .. rb+RPC+HALO+1 (for conv we need rb-1..rb+32)
        pad = np.zeros((RIN, WP, C), np.float32)
        lo = rb - HALO
        hi = rb + RPC + HALO
        slo, shi = max(lo, 0), min(hi, H)
        pad[slo - lo : shi - lo, GUARD : GUARD + W] = x[b, slo:shi]
        x_rows_h = np.ascontiguousarray(
            pad.reshape(RIN * WP, C).astype(ml_dtypes.bfloat16)
        )
        # conv x_T: rows rb-1 .. rb+32 (34), cols -1..64 (66), ch-on-partition
        conv_rows = pad[HALO - 1 : HALO - 1 + CT, GUARD - 1 : GUARD - 1 + CW]
        x_t = np.transpose(conv_rows, (2, 0, 1)).reshape(C, CT * CW)
        x_t16 = x_t.astype(ml_dtypes.bfloat16)
        in_maps.append(
            dict(
                x_rows=x_rows_h,
                x_t0=np.ascontiguousarray(x_t16[0:128]),
                x_t1=np.ascontiguousarray(x_t16[128:256]),
                offw=offw_h,
                dcnw=dcnw_h,
                bn=bn_h,
                idxb=idxb8,
                ident=ident,
                identf=identf,
            )
        )
    return in_maps


def kernel(x, offset_w, dcn_w, gamma, beta, moving_mean, moving_var):
    in_maps = make_in_maps(
        x, offset_w, dcn_w, gamma, beta, moving_mean, moving_var
    )
    nc = build_graph()
    res = run_bass_kernel_spmd(nc, in_maps, list(range(NCORES)))
    outs = res.results if hasattr(res, "results") else res

    full = np.zeros((B, H, W, F), np.float32)
    for core in range(NCORES):
        o = np.asarray(outs[core]["out"]).astype(np.float32)  # [2, 128, P]
        o = o.reshape(256, P).T.reshape(RPC, W, F)
        r0 = core * RPC
        full[r0 // H, r0 % H : r0 % H + RPC] = o
    return full


if __name__ == "__main__":
    import reference

    inp = {k: np.asarray(v) for k, v in reference.setup_inputs().items()}
    got = kernel(**inp)
    print("kernel ran, shape", got.shape)
